# revision 31
# baseline (speedup 1.0000x reference)
"""Trainium2 Bass kernel: 1024-point FFT of real rows -> (real, imag).

Math: out = FFT_1024(x[b, :]) per row. Two folding levels over the real
input x (U[n] = x[n]+x[1024-n], V[n] = x[n]-x[1024-n]), then a radix-2
split of the half-spectrum k in [1,513) by parity:
  Xr[2m]   = sum_{n<256} Aue[n] cos(2pi n m/512)   + U[256](-1)^m
  Xi[2m]   = sum_{n<256} Avo[n] (-sin(2pi n m/512))
  Xr[2m+1] = sum_{n<256} Auo[n] cos(pi n(2m+1)/512)
  Xi[2m+1] = sum_{n<256} Ave[n] (-sin(pi n(2m+1)/512)) - V[256](-1)^m
with Aue/Auo/Avo/Ave the second-level even/odd folds of U and V. The
remaining half follows from X[1024-k] = conj(X[k]); k=0 is a row sum.

The device computes the four quadrants in TRANSPOSED orientation (freq
on PSUM partitions, batch on the free dim): per 512-row group and
128-wide k-tile, K=256 fp16 matmuls (2 accumulating chunks) with the
quarter-size cos/sin matrices as the stationary operand — 16 matmuls
per group. Rank-1 edge terms ride dead coefficient rows (row 0 of
Aue/Auo absorbs U[0] +/- x[512] via all-ones coefficient rows; row 0
of Ave carries V[256] via Coi's overridden row 0) except the even-real
U[256](-1)^m term, which the host adds during assembly.

All device I/O is fp16 (the correctness gate is rel-err < 2e-2; fp16
quantization contributes ~5e-4), halving HBM traffic vs fp32 — the
kernel is DMA-bound. Input tensors are partition-major so every DMA
moves one long contiguous run per partition (8 KiB for the fold data),
and all four group loads prefetch at program start. The host performs
the pure data-expansion assembly: parity interleave, conjugate mirror,
k=0 column, U[256] rank-1 add, final layout transpose. Inputs ride the
sync queue, real/imag outputs the gpsimd/scalar queues. Pure
data-parallel across 8 cores, no collectives.
"""

import os
import numpy as np

N_FFT = 1024
BATCH = 16384
N_CORES = 8
B_CORE = BATCH // N_CORES  # 2048
P = 128
HALF = 512
QU = 256
GC = 512                   # batch rows per group
N_WARM = 12                # HAM warmup matmuls (cover input stream-in)
CSCALE = 127.0             # int8 coefficient scale (host divides it back out)

_BUILD_CACHE = {}


def _constants():
    """Coefficient block [P, 4, 2, QU] int8 (x127, partition-major)."""
    n = np.arange(QU, dtype=np.float64)[:, None]
    c = np.arange(QU, dtype=np.float64)[None, :]
    cer = np.cos(2 * np.pi * n * (c + 1) / 512)       # row 0 = 1 (absorbs U0+x512)
    cei = -np.sin(2 * np.pi * n * (c + 1) / 512)      # row 0 = 0 (kills U[256] slot)
    cor = np.cos(np.pi * n * (2 * c + 1) / 512)       # row 0 = 1 (absorbs U0-x512)
    coi = -np.sin(np.pi * n * (2 * c + 1) / 512)
    coi[0, :] = np.where(np.arange(QU) % 2 == 0, -1.0, 1.0)  # carries -V[256](-1)^m
    coef = np.empty((P, 4, 2, QU), dtype=np.int8)
    for ci, m in enumerate((cer, cei, cor, coi)):
        q = np.rint(m * CSCALE).astype(np.int8)
        coef[:, ci] = q.reshape(P, 2, QU)
    return coef


def build_nc(b_core=B_CORE):
    """Build + compile the per-core Bass program (same NEFF on all cores)."""
    import concourse.mybir as mybir
    import concourse.tile as tile
    from concourse import bacc

    f32 = mybir.dt.float32
    f16 = mybir.dt.float16
    i8 = mybir.dt.int8

    gc = min(GC, b_core)
    n_groups = b_core // gc
    n_kt = QU // P             # 128-wide k-tiles per quadrant (2)

    nc = bacc.Bacc(
        "TRN2", target_bir_lowering=False, debug=False, num_devices=N_CORES
    )

    # partition-major interleaved folds: group g / partition p / fold a /
    # chunk j / batch b, holding data row n = 2p+j of fold a
    data_in = nc.dram_tensor(
        "data", [n_groups, P, 4, 2, gc], f16, kind="ExternalInput"
    )
    coef_in = nc.dram_tensor("coef", [P, 4, 2, QU], i8, kind="ExternalInput")
    # transposed halves, group-blocked: row r = 4p + slot;
    # slots 0,1 = even-k tiles, 2,3 = odd-k tiles (host interleaves)
    o_rt = nc.dram_tensor("o_rt", [n_groups, HALF, gc], f16, kind="ExternalOutput")
    o_it = nc.dram_tensor("o_it", [n_groups, HALF, gc], f16, kind="ExternalOutput")

    data_r = data_in.ap()
    ort_r = o_rt.ap().rearrange("g (p t) b -> g p t b", t=4)
    oit_r = o_it.ap().rearrange("g (p t) b -> g p t b", t=4)
    CER, CEI, COR, COI = 0, 1, 2, 3
    AUE, AVO, AUO, AVE = 0, 1, 2, 3    # fold slot order

    with tile.TileContext(nc) as tc:
        with (
            tc.tile_pool(name="const", bufs=1) as cpool,
            tc.tile_pool(name="work", bufs=n_groups) as wpool,
            tc.tile_pool(name="outp", bufs=3) as opool,
            tc.tile_pool(name="psm", bufs=4, space="PSUM") as psm,
        ):
            # coef loads first on the sync ring: small, and it gates the
            # first matmuls (the ACT ring is slow to start streaming)
            coef_i8 = cpool.tile([P, 4, 2, QU], i8, name="coef_i8")
            nc.sync.dma_start(out=coef_i8[:], in_=coef_in.ap())

            # prefetch ALL group loads up front (one 1 MiB DMA per group,
            # 8 KiB contiguous per partition)
            dat = []
            for g in range(n_groups):
                dg = wpool.tile([P, 4, 2, gc], f16, tag="dat", name=f"dat_{g}")
                nc.sync.dma_start(out=dg[:], in_=data_r[g])
                dat.append(dg)

            # HAM warmup: keep the PE busy on a zeroed tile while inputs
            # stream in (borrows a "pr" psum slot; released untouched)
            wz = cpool.tile([P, HALF], f16, name="warm_zero")
            nc.vector.memset(wz[:], 0.0)
            wu = psm.tile([P, gc], f32, tag="pr")
            for w in range(N_WARM):
                nc.tensor.matmul(
                    wu[:], lhsT=wz[:, 0:P], rhs=wz[:, 0:gc],
                    start=(w == 0), stop=(w == N_WARM - 1),
                )

            # upcast int8 coefficients (x127) to fp16 for the PE
            coef_sb = cpool.tile([P, 4, 2, QU], f16, name="coef")
            nc.vector.tensor_copy(out=coef_sb[:], in_=coef_i8[:])

            for g in range(n_groups):
                ortg = opool.tile([P, 4, gc], f16, tag="ortg")
                oitg = opool.tile([P, 4, gc], f16, tag="oitg")

                for kt in range(n_kt):
                    ksl = slice(kt * P, (kt + 1) * P)
                    quads = (
                        ("pr", CER, AUE, 2 * kt, "v"),   # even-real
                        ("pi", CEI, AVO, 2 * kt, "s"),   # even-imag
                        ("pr", COR, AUO, 2 * kt + 1, "v"),  # odd-real
                        ("pi", COI, AVE, 2 * kt + 1, "s"),  # odd-imag
                    )
                    for tag, ci, ai, slot, eng in quads:
                        ps = psm.tile([P, gc], f32, tag=tag)
                        for j in range(2):
                            nc.tensor.matmul(
                                ps[:], lhsT=coef_sb[:, ci, j, ksl],
                                rhs=dat[g][:, ai, j], start=(j == 0), stop=(j == 1),
                            )
                        # PSUM fp32 -> SBUF fp16 (cast on copy)
                        if eng == "v":
                            nc.vector.tensor_copy(out=ortg[:, slot], in_=ps[:])
                        else:
                            nc.scalar.copy(out=oitg[:, slot], in_=ps[:])

                    # drain per k-tile: 2 KiB contiguous per-partition runs;
                    # starts the output stream as early as possible
                    ksl2 = slice(2 * kt, 2 * kt + 2)
                    nc.gpsimd.dma_start(out=ort_r[g][:, ksl2], in_=ortg[:, ksl2])
                    nc.scalar.dma_start(out=oit_r[g][:, ksl2], in_=oitg[:, ksl2])

    nc.compile()
    return nc


def _get_nc(b_core=B_CORE):
    if b_core not in _BUILD_CACHE:
        _BUILD_CACHE[b_core] = build_nc(b_core)
    return _BUILD_CACHE[b_core]


def _host_prep(x):
    """Two-level real-FFT folds (transposed) + host-side edge terms."""
    B = x.shape[0]
    U = np.empty((B, HALF), dtype=np.float32)
    V = np.empty((B, HALF), dtype=np.float32)
    U[:, 0] = x[:, 0]
    rev = x[:, 1023:HALF:-1]
    np.add(x[:, 1:HALF], rev, out=U[:, 1:HALF])
    np.subtract(x[:, 1:HALF], rev, out=V[:, 1:HALF])
    x512 = x[:, HALF]
    a = {k: np.empty((B, QU), dtype=np.float32)
         for k in ("aue", "auo", "avo", "ave")}
    a["aue"][:, 0] = U[:, 0] + x512
    a["auo"][:, 0] = U[:, 0] - x512
    a["avo"][:, 0] = 0.0                       # U[256] slot handled on host
    a["ave"][:, 0] = V[:, QU]                  # = x[256] - x[768]
    urev = U[:, 511:QU:-1]
    vrev = V[:, 511:QU:-1]
    np.add(U[:, 1:QU], urev, out=a["aue"][:, 1:QU])
    np.subtract(U[:, 1:QU], urev, out=a["auo"][:, 1:QU])
    np.subtract(V[:, 1:QU], vrev, out=a["avo"][:, 1:QU])
    np.add(V[:, 1:QU], vrev, out=a["ave"][:, 1:QU])
    col0 = (U.sum(axis=1, dtype=np.float64) + x512).astype(np.float32)
    u256 = U[:, QU].copy()                     # = x[256] + x[768]
    at = {k: np.ascontiguousarray(v.T).astype(np.float16) for k, v in a.items()}
    return at, col0, u256


def _blocked(at, sl, b_core):
    """fold dict of [256, B] -> partition-major [n_groups, P, 4, 2, gc]."""
    gc = min(GC, b_core)
    n_groups = b_core // gc
    out = np.empty((n_groups, P, 4, 2, gc), dtype=np.float16)
    for a, k in enumerate(("aue", "avo", "auo", "ave")):
        s = at[k][:, sl]                       # [256, b_core] fp16
        # row n = 2p+j -> [g, p, j, b]
        out[:, :, a] = s.reshape(P, 2, n_groups, gc).transpose(2, 0, 1, 3)
    return out


def _assemble(half_t, out, sl, b_core, u256, neg_mirror):
    """Device half [n_groups, 512(r=4p+slot), gc] -> out[sl, :] (1024 cols).

    slot 0,1: even k = 2*(kt*128 + p + 1); slot 2,3: odd k = 2*(kt*128+p)+1.
    """
    gc = min(GC, b_core)
    n_groups = b_core // gc
    h = half_t.reshape(n_groups, P, 4, gc)
    b0 = sl.start
    for g in range(n_groups):
        rows = slice(b0 + g * gc, b0 + (g + 1) * gc)
        for kt in range(2):
            e0 = 2 * (kt * P) + 2
            out[rows, e0 : e0 + 2 * P : 2] = h[g, :, 2 * kt, :].T
            o0 = 2 * (kt * P) + 1
            out[rows, o0 : o0 + 2 * P : 2] = h[g, :, 2 * kt + 1, :].T
    blk = out[sl]
    blk[:, 1:513] *= np.float32(1.0 / CSCALE)   # undo int8 coefficient scale
    if u256 is not None:
        # Xr[2m] += U[256]*(-1)^m for m=1..256 (k = 2..512 even)
        sign = np.where(np.arange(1, QU + 1) % 2 == 1, -1.0, 1.0).astype(np.float32)
        blk[:, 2:513:2] += u256[sl, None] * sign[None, :]
    if neg_mirror:
        np.negative(blk[:, 511:0:-1], out=blk[:, 513:1024])
    else:
        blk[:, 513:1024] = blk[:, 511:0:-1]


def kernel(**inputs):
    from concourse.bass_utils import run_bass_kernel_spmd

    x = np.ascontiguousarray(np.asarray(inputs["x"], dtype=np.float32))
    assert x.shape == (BATCH, N_FFT), x.shape
    coef = _constants()
    at, col0, u256 = _host_prep(x)
    nc = _get_nc()
    in_maps = []
    for c in range(N_CORES):
        sl = slice(c * B_CORE, (c + 1) * B_CORE)
        m = {"data": _blocked(at, sl, B_CORE), "coef": coef}
        in_maps.append(m)
    trace = bool(int(os.environ.get("FFT_KERNEL_TRACE", "0")))
    try:
        res = run_bass_kernel_spmd(
            nc, in_maps, core_ids=list(range(N_CORES)), trace=trace
        )
    except Exception:
        # transient NRT/device hiccups have been observed; retry once
        res = run_bass_kernel_spmd(
            nc, in_maps, core_ids=list(range(N_CORES)), trace=trace
        )
    if trace:
        kernel.last_results = res
    real = np.empty((BATCH, N_FFT), dtype=np.float32)
    imag = np.empty((BATCH, N_FFT), dtype=np.float32)
    for c in range(N_CORES):
        sl = slice(c * B_CORE, (c + 1) * B_CORE)
        _assemble(res.results[c]["o_rt"], real, sl, B_CORE, u256, neg_mirror=False)
        _assemble(res.results[c]["o_it"], imag, sl, B_CORE, None, neg_mirror=True)
    real[:, 0] = col0
    imag[:, 0] = 0.0
    return real, imag
